# revision 1
# baseline (speedup 1.0000x reference)
"""Trainium2 Bass kernel for nn_CIntegration_3487513444382 (embedding_lookup).

Computation (per token): ct = concat(onehot(rgap,32), onehot(sgap,32),
onehot(pcount,32)); out = concat(vt * (ct @ W.T), ct).

Strategy: pure data parallel over the batch dim (64 -> 8 per core).
Per core, tokens are laid out p-major (token t -> partition t//64, slot
t%64) so every DMA moves large contiguous per-partition runs. The
gather ct @ W.T runs on the PE as a one-hot matmul: a tiny E3 matmul
broadcasts the (offset) indices across 96 partitions, a DVE compare
against an iota column builds the transposed one-hot in bf16 (exact),
and W.T is applied as a hi+lo bf16 split accumulated in fp32 PSUM
(~1e-5 absolute error on unit-scale outputs). The token-major one-hot
for the output tail is built by a second DVE compare, and the vt gate
is a single fp32 DVE multiply per 4-chunk quad.
"""
import numpy as np

import concourse.bass as bass
import concourse.tile as tile
from concourse import bacc, mybir
from concourse.bass_utils import run_bass_kernel_spmd

F32 = mybir.dt.float32
BF16 = mybir.dt.bfloat16

N_CORES = 8
B, S, E = 64, 1024, 256
BPC = B // N_CORES          # 8 batches per core
NTOK = BPC * S              # 8192 tokens per core
NCH = NTOK // 128           # 64 chunks of 128 tokens
G = 4                       # chunks per DMA group / compute quad
NGRP = NCH // G             # 16 groups
NTOT = 96                   # one-hot width
OUTW = E + NTOT             # 352

_NC = None


def _build_nc():
    nc = bacc.Bacc("TRN2", target_bir_lowering=False, debug=False,
                   num_devices=N_CORES)
    vt = nc.dram_tensor("vt", [NTOK, E], F32, kind="ExternalInput")
    idx = nc.dram_tensor("idx", [128, NCH, 3], F32, kind="ExternalInput")
    idxt = nc.dram_tensor("idxt", [3, NTOK], BF16, kind="ExternalInput")
    wt_hi = nc.dram_tensor("wt_hi", [NTOT, E], BF16, kind="ExternalInput")
    wt_lo = nc.dram_tensor("wt_lo", [NTOT, E], BF16, kind="ExternalInput")
    out = nc.dram_tensor("out", [NTOK, OUTW], F32, kind="ExternalOutput")

    with tile.TileContext(nc) as tc:
        with (
            tc.tile_pool(name="const", bufs=1) as const,
            tc.tile_pool(name="vtp", bufs=10) as vtp,
            tc.tile_pool(name="outp", bufs=8) as outp,
            tc.tile_pool(name="ctt", bufs=3) as ctt,
            tc.tile_pool(name="bcs", bufs=3) as bcs,
            tc.tile_pool(name="ps_b", bufs=2, space="PSUM") as ps_b,
            tc.tile_pool(name="ps_m", bufs=3, space="PSUM") as ps_m,
        ):
            # token layout views (needed for the early first load)
            vt_view = vt.ap().rearrange("(p i) e -> p i e", p=128)
            out_view = out.ap().rearrange("(p i) f -> p i f", p=128)
            # index consts first: they gate the whole compute front-end,
            # and they are small, so completing them before the big vt
            # packets lets PE/ACT/DVE run during the load stream
            idxt_sb = const.tile([3, NTOK], BF16)
            nc.sync.dma_start(idxt_sb[:], idxt.ap())
            idx_sb = const.tile([128, NCH, 3], F32)
            nc.sync.dma_start(idx_sb[:], idx.ap())
            # group-0 load next on the fast HWDGE ring
            vt_big0 = vtp.tile([128, G, E], F32, tag="vt_big")
            nc.sync.dma_start(vt_big0[:], vt_view[:, 0:G, :])
            wth_sb = const.tile([NTOT, E], BF16)
            nc.sync.dma_start(wth_sb[:], wt_hi.ap())
            wtl_sb = const.tile([NTOT, E], BF16)
            nc.sync.dma_start(wtl_sb[:], wt_lo.ap())
            # device-built constants (no DMA: tiny loads would be starved
            # behind the big vt packets on the shared SDMA engines)
            e3_sb = const.tile([3, NTOT], BF16)
            nc.gpsimd.memset(e3_sb[:], 1.0)
            nc.gpsimd.affine_select(
                out=e3_sb[:].rearrange("p (a b) -> p a b", a=3),
                in_=e3_sb[:].rearrange("p (a b) -> p a b", a=3),
                pattern=[[1, 3], [0, 32]],
                compare_op=mybir.AluOpType.is_equal,
                fill=0.0, base=0, channel_multiplier=-1,
            )
            iota_row = const.tile([128, NTOT], F32)
            nc.gpsimd.iota(iota_row[:], [[1, NTOT]], channel_multiplier=0,
                           allow_small_or_imprecise_dtypes=True)
            iota_col = const.tile([NTOT, 1], F32)
            nc.gpsimd.iota(iota_col[:], [[0, 1]], channel_multiplier=1,
                           allow_small_or_imprecise_dtypes=True)

            for g in range(NGRP):
                if g == 0:
                    vt_big = vt_big0
                else:
                    vt_big = vtp.tile([128, G, E], F32, tag="vt_big")
                    nc.gpsimd.dma_start(
                        vt_big[:], vt_view[:, g * G:(g + 1) * G, :])
                out_big = outp.tile([128, G, OUTW], F32)

                cq = g * G
                # broadcast idx rows for the quad: bc[96,512] = E3.T @ idxT
                bc_ps = ps_b.tile([NTOT, 4 * 128], F32)
                nc.tensor.matmul(
                    bc_ps[:], e3_sb[:],
                    idxt_sb[:, cq * 128:(cq + 4) * 128],
                    start=True, stop=True,
                )
                # PSUM->SBUF on the idle Scalar engine, then the compare
                # runs in DVE 2x mode (fp32 tensor_scalar from SBUF)
                bc_sb = bcs.tile([NTOT, 4 * 128], F32)
                nc.scalar.copy(bc_sb[:], bc_ps[:])
                ct_t = ctt.tile([NTOT, 4 * 128], BF16)
                nc.vector.tensor_scalar(
                    ct_t[:], bc_sb[:], iota_col[:, 0:1], None,
                    mybir.AluOpType.is_equal,
                )
                mm_ps = ps_m.tile([128, 4, E], F32)
                for k in range(4):
                    lhs = ct_t[:, k * 128:(k + 1) * 128]
                    # Cct chunk = ct @ (W_hi + W_lo).T, PSUM-accumulated
                    nc.tensor.matmul(mm_ps[:, k, :], lhs, wth_sb[:],
                                     start=True, stop=False)
                    nc.tensor.matmul(mm_ps[:, k, :], lhs, wtl_sb[:],
                                     start=False, stop=True)
                # token-major one-hot for the whole quad
                nc.vector.tensor_tensor(
                    out_big[:, :, E:OUTW].rearrange(
                        "p c (j k) -> p c j k", j=3),
                    iota_row[:, None, :].broadcast_to(
                        [128, G, NTOT]).rearrange(
                        "p c (j k) -> p c j k", j=3),
                    idx_sb[:, cq:cq + G, :, None].broadcast_to(
                        [128, G, 3, 32]),
                    mybir.AluOpType.is_equal,
                )
                if g < NGRP - 2:
                    # theta = vt * Cct for the quad, one DVE op
                    nc.vector.tensor_tensor(
                        out_big[:, :, 0:E],
                        vt_big[:],
                        mm_ps[:],
                        mybir.AluOpType.mult,
                    )
                    nc.sync.dma_start(
                        out_view[:, g * G:(g + 1) * G, :], out_big[:])
                else:
                    # endgame: pair-sized muls + stores so the final store
                    # is small and starts as early as possible
                    for h in range(2):
                        nc.vector.tensor_tensor(
                            out_big[:, 2 * h:2 * h + 2, 0:E],
                            vt_big[:, 2 * h:2 * h + 2, :],
                            mm_ps[:, 2 * h:2 * h + 2, :],
                            mybir.AluOpType.mult,
                        )
                        nc.sync.dma_start(
                            out_view[:, g * G + 2 * h:g * G + 2 * h + 2, :],
                            out_big[:, 2 * h:2 * h + 2, :])

    nc.compile()
    return nc


def _get_nc():
    global _NC
    if _NC is None:
        _NC = _build_nc()
    return _NC


def _host_prep(vt, rgap, sgap, pcount, W):
    import ml_dtypes
    bf16 = ml_dtypes.bfloat16
    vt = np.asarray(vt, dtype=np.float32)
    rgap = np.asarray(rgap)
    sgap = np.asarray(sgap)
    pcount = np.asarray(pcount)
    W = np.asarray(W, dtype=np.float32)
    wt = np.ascontiguousarray(W.T)              # [96, 256]
    wt_hi = wt.astype(bf16)
    wt_lo = (wt - wt_hi.astype(np.float32)).astype(bf16)
    in_maps = []
    for m in range(N_CORES):
        sl = slice(m * BPC, (m + 1) * BPC)
        vts = np.ascontiguousarray(vt[sl].reshape(NTOK, E))
        idxs = np.stack(
            [rgap[sl].reshape(NTOK),
             sgap[sl].reshape(NTOK) + 32,
             pcount[sl].reshape(NTOK) + 64], axis=-1
        ).astype(np.float32)                    # [8192, 3]
        # token t = p*64 + i: idx[p, i, j]; idxt columns chunk-major (i*128+p)
        idx_arr = np.ascontiguousarray(idxs.reshape(128, NCH, 3))
        idxt = np.ascontiguousarray(
            idxs.reshape(128, NCH, 3).transpose(2, 1, 0).reshape(3, NTOK)
        ).astype(bf16)                          # [3, 8192]
        in_maps.append({"vt": vts, "idx": idx_arr, "idxt": idxt,
                        "wt_hi": wt_hi, "wt_lo": wt_lo})
    return in_maps


def kernel(vt, rgap, sgap, pcount, W, _trace=False, _tmpdir=None):
    nc = _get_nc()
    in_maps = _host_prep(vt, rgap, sgap, pcount, W)
    res = run_bass_kernel_spmd(
        nc, in_maps, list(range(N_CORES)),
        trace=_trace, **({"tmpdir": _tmpdir} if _tmpdir else {}),
    )
    outs = [res.results[m]["out"].reshape(BPC, S, OUTW) for m in range(N_CORES)]
    full = np.concatenate(outs, axis=0).astype(np.float32, copy=False)
    if _trace:
        return full, res
    return full



# revision 2
# speedup vs baseline: 1.4118x; 1.4118x over previous
"""Trainium2 Bass kernel for nn_CIntegration_3487513444382 (embedding_lookup).

Computation (per token): ct = concat(onehot(rgap,32), onehot(sgap,32),
onehot(pcount,32)); out = concat(vt * (ct @ W.T), ct).

Strategy: pure data parallel over the batch dim (64 -> 8 per core), with
all device-side tensors transposed to [feature, token] so the rel-err
budget (2e-2) can buy bandwidth: vt is fed as bf16, theta is stored as
bf16, and the one-hot tail is stored as fp8 (0/1 exact). Per core this
moves ~9.3MB instead of ~20MB of f32 traffic.

Per 512-token group: a tiny E3 matmul broadcasts the (offset) indices
to 96 partitions, a DVE compare against an iota column builds the
transposed one-hot directly in fp8, the PE streams that one-hot through
stationary W.T halves (bf16) to produce Cct.T in PSUM, ScalarE copies
it to SBUF as bf16, and a 2x-mode DVE multiply applies the vt gate.
The fp8 one-hot doubles as the stored ct output (transposed; the host
un-transposes and upcasts).
"""
import numpy as np

import concourse.bass as bass
import concourse.tile as tile
from concourse import bacc, mybir
from concourse.bass_utils import run_bass_kernel_spmd

F32 = mybir.dt.float32
BF16 = mybir.dt.bfloat16
FP8 = mybir.dt.float8e4

N_CORES = 8
B, S, E = 64, 1024, 256
BPC = B // N_CORES          # 8 batches per core
NTOK = BPC * S              # 8192 tokens per core
NCH = NTOK // 128           # 64 chunks of 128 tokens
G = 4                       # chunks per compute group
NGRP = NCH // G             # 16 groups of 512 tokens
GTOK = G * 128              # 512
NTOT = 96                   # one-hot width
EH = E // 128               # 2 e-halves

_NC = None


def _build_nc():
    nc = bacc.Bacc("TRN2", target_bir_lowering=False, debug=False,
                   num_devices=N_CORES)
    vtT = nc.dram_tensor("vtT", [E, NTOK], BF16, kind="ExternalInput")
    idxt = nc.dram_tensor("idxt", [3, NTOK], BF16, kind="ExternalInput")
    wt = nc.dram_tensor("wt", [NTOT, E], BF16, kind="ExternalInput")
    thetaT = nc.dram_tensor("thetaT", [E, NTOK], BF16, kind="ExternalOutput")
    ctT = nc.dram_tensor("ctT", [NTOT, NTOK], FP8, kind="ExternalOutput")

    with tile.TileContext(nc) as tc:
        with (
            tc.tile_pool(name="const", bufs=1) as const,
            tc.tile_pool(name="vtp", bufs=4) as vtp,
            tc.tile_pool(name="outp", bufs=3) as outp,
            tc.tile_pool(name="ctp", bufs=2) as ctp,
            tc.tile_pool(name="mmsb", bufs=3) as mmsb,
            tc.tile_pool(name="ps_b", bufs=2, space="PSUM") as ps_b,
            tc.tile_pool(name="ps_m", bufs=3, space="PSUM") as ps_m,
        ):
            # [e, tok] views split the 256 e-rows into 2 x 128 partitions
            vt_view = vtT.ap().rearrange("(h p) t -> p h t", p=128)
            th_view = thetaT.ap().rearrange("(h p) t -> p h t", p=128)
            # small consts first on the fast HWDGE ring: they gate the
            # whole compute front-end
            idxt_sb = const.tile([3, NTOK], BF16)
            nc.sync.dma_start(idxt_sb[:], idxt.ap())
            wt_sb = const.tile([NTOT, E], BF16)
            nc.sync.dma_start(wt_sb[:], wt.ap())
            # first vt pair load next, still on HWDGE
            vt0 = vtp.tile([128, EH, 2 * GTOK], BF16, tag="vt")
            nc.sync.dma_start(vt0[:], vt_view[:, :, 0:2 * GTOK])
            # device-built constants (no DMA)
            e3_sb = const.tile([3, NTOT], BF16)
            nc.gpsimd.memset(e3_sb[:], 1.0)
            nc.gpsimd.affine_select(
                out=e3_sb[:].rearrange("p (a b) -> p a b", a=3),
                in_=e3_sb[:].rearrange("p (a b) -> p a b", a=3),
                pattern=[[1, 3], [0, 32]],
                compare_op=mybir.AluOpType.is_equal,
                fill=0.0, base=0, channel_multiplier=-1,
            )
            iota_col = const.tile([NTOT, 1], F32)
            nc.gpsimd.iota(iota_col[:], [[0, 1]], channel_multiplier=1,
                           allow_small_or_imprecise_dtypes=True)
            # prefetch two more vt pairs on SWDGE
            vt_tiles = [vt0]
            for pair in (1, 2):
                t = vtp.tile([128, EH, 2 * GTOK], BF16, tag="vt")
                nc.gpsimd.dma_start(
                    t[:], vt_view[:, :, pair * 2 * GTOK:(pair + 1) * 2 * GTOK])
                vt_tiles.append(t)

            th_tile = None
            ct8 = None
            for g in range(NGRP):
                pair, sub = g // 2, g % 2
                if sub == 0:
                    if pair + 3 < NGRP // 2:
                        t = vtp.tile([128, EH, 2 * GTOK], BF16, tag="vt")
                        nc.gpsimd.dma_start(
                            t[:], vt_view[:, :, (pair + 3) * 2 * GTOK:
                                          (pair + 4) * 2 * GTOK])
                        vt_tiles.append(t)
                    th_tile = outp.tile([128, EH, 2 * GTOK], BF16, tag="th")
                if g % 4 == 0:
                    ct8 = ctp.tile([NTOT, 4 * GTOK], FP8, tag="ct8")
                q = g % 4
                vt_big = vt_tiles[pair]

                # bc[96, 512] = E3.T @ idxT: index value broadcast per segment
                bc_ps = ps_b.tile([NTOT, GTOK], F32)
                nc.tensor.matmul(
                    bc_ps[:], e3_sb[:],
                    idxt_sb[:, g * GTOK:(g + 1) * GTOK],
                    start=True, stop=True,
                )
                # transposed one-hot, built once, directly in fp8 (exact)
                ct_sl = ct8[:, q * GTOK:(q + 1) * GTOK]
                nc.vector.tensor_scalar(
                    ct_sl, bc_ps[:], iota_col[:, 0:1], None,
                    mybir.AluOpType.is_equal,
                )
                # Cct.T halves: stationary W.T half, stream the one-hot
                mm_ps = ps_m.tile([128, EH, GTOK], F32)
                for h in range(EH):
                    nc.tensor.matmul(
                        mm_ps[:, h, :], wt_sb[:, h * 128:(h + 1) * 128],
                        ct_sl, start=True, stop=True,
                    )
                # PSUM -> SBUF bf16 on the otherwise idle Scalar engine so
                # the gate runs in DVE 2x mode
                mm_sb = mmsb.tile([128, EH, GTOK], BF16)
                nc.scalar.copy(mm_sb[:], mm_ps[:])
                # theta.T = vt.T * Cct.T (bf16 2x-mode multiply)
                nc.vector.tensor_tensor(
                    th_tile[:, :, sub * GTOK:(sub + 1) * GTOK],
                    vt_big[:, :, sub * GTOK:(sub + 1) * GTOK],
                    mm_sb[:],
                    mybir.AluOpType.mult,
                )
                if sub == 1:
                    nc.sync.dma_start(
                        th_view[:, :, pair * 2 * GTOK:(pair + 1) * 2 * GTOK],
                        th_tile[:])
                if q == 3:
                    base = (g - 3) * GTOK
                    nc.sync.dma_start(
                        ctT.ap()[:, base:base + 4 * GTOK], ct8[:])

    nc.compile()
    return nc


def _get_nc():
    global _NC
    if _NC is None:
        _NC = _build_nc()
    return _NC


def _host_prep(vt, rgap, sgap, pcount, W):
    import ml_dtypes
    bf16 = ml_dtypes.bfloat16
    vt = np.asarray(vt, dtype=np.float32)
    rgap = np.asarray(rgap)
    sgap = np.asarray(sgap)
    pcount = np.asarray(pcount)
    W = np.asarray(W, dtype=np.float32)
    wt = np.ascontiguousarray(W.T).astype(bf16)     # [96, 256]
    in_maps = []
    for m in range(N_CORES):
        sl = slice(m * BPC, (m + 1) * BPC)
        # token t = p*64 + i maps to column tau = i*128 + p
        idxs = np.stack(
            [rgap[sl].reshape(NTOK),
             sgap[sl].reshape(NTOK) + 32,
             pcount[sl].reshape(NTOK) + 64], axis=0)          # [3, t]
        idxt = np.ascontiguousarray(
            idxs.reshape(3, 128, NCH).transpose(0, 2, 1).reshape(3, NTOK)
        ).astype(bf16)
        vtT = np.ascontiguousarray(
            vt[sl].reshape(128, NCH, E).transpose(2, 1, 0).reshape(E, NTOK)
        ).astype(bf16)
        in_maps.append({"vtT": vtT, "idxt": idxt, "wt": wt})
    return in_maps


def kernel(vt, rgap, sgap, pcount, W, _trace=False, _tmpdir=None):
    nc = _get_nc()
    in_maps = _host_prep(vt, rgap, sgap, pcount, W)
    res = run_bass_kernel_spmd(
        nc, in_maps, list(range(N_CORES)),
        trace=_trace, **({"tmpdir": _tmpdir} if _tmpdir else {}),
    )
    full = np.empty((B, S, E + NTOT), dtype=np.float32)
    for m in range(N_CORES):
        sl = slice(m * BPC, (m + 1) * BPC)
        view = full[sl].reshape(NTOK, E + NTOT)
        thetaT = np.asarray(res.results[m]["thetaT"]).astype(np.float32)
        ct8 = np.asarray(res.results[m]["ctT"]).astype(np.float32)
        view[:, :E] = thetaT.reshape(E, NCH, 128).transpose(2, 1, 0) \
                            .reshape(NTOK, E)
        view[:, E:] = ct8.reshape(NTOT, NCH, 128).transpose(2, 1, 0) \
                         .reshape(NTOK, NTOT)
    if _trace:
        return full, res
    return full
